# revision 1
# baseline (speedup 1.0000x reference)
"""ConvCNP2D Trainium2 kernel.

Strategy (8 cores, uniform SPMD program; per-core data selects work):
  core c -> batch b = c//2, grid i-half h = c%2.

The grid is a product grid (meshgrid of two 1D linspaces), so every RBF
set-conv separates:  K[n,m] = E1x[n,i_m] * E1y[n,j_m]   (stage 1) and
K2[n,m] = E[i_n,i_m] * F[j_n,j_m] with E,F only 32x32 (stage 3). This
removes the (b,n,m,C) materialization entirely: stage 1 is one 256-
contraction matmul, stage 3 is two 32-contraction matmuls on each side
of a 16-channel projection that is folded in FIRST (W_m/W_s are (1,16),
so project h -> (2,1024) before the grid transforms).

The 5x5 convs run as 5 accumulated matmuls over (ci*5row-shift) packed
partitions; the packs are pure DMA row-shifted copies (f32r views) so
TensorE streams at 1 cyc/row (f32r) instead of fp32's 4.

Partition-crossing reshapes (free dim -> partition dim) ride through
small DRAM staging buffers (DMA-only, ~KBs).
"""
import sys

for _p in ('/opt/trn_rl_repo', '/root/.axon_site/_ro/trn_rl_repo'):
    if _p not in sys.path:
        sys.path.append(_p)

import numpy as np
import concourse.bass as bass
import concourse.tile as tile
from concourse import mybir
from concourse.bass_utils import run_bass_kernel_spmd
from concourse.vector_clock import ScopedClock
from contextlib import ExitStack

F32 = mybir.dt.float32
F32R = mybir.dt.float32r
AF = mybir.ActivationFunctionType

GRID, B, N, CIN, CHID, COUT = 32, 4, 256, 16, 32, 16
NCORES = 8
WAIT_LIMIT = 1


class _TC(tile.TileContext):
    """This walrus build accepts only 1 sync wait per instruction; split
    the end-of-kernel drain's global-clock waits across NOPs."""

    def _drain_and_barrier(self, tick_clock, wait_clock):
        nc = self.nc
        collector = nc.sync.nop()
        wait_clock.add_sem_waits(
            collector.ins, ScopedClock({None: tick_clock.global_clock}))
        si = collector.ins.sync_info
        waits = list(si.on_wait) if si and si.on_wait else []
        if len(waits) > WAIT_LIMIT:
            si.on_wait = waits[:WAIT_LIMIT]
            for i in range(WAIT_LIMIT, len(waits), WAIT_LIMIT):
                extra = nc.sync.nop()
                xsi = extra.ins.sync_info
                if xsi is None:
                    extra.ins.sync_info = mybir.SyncInfo(
                        on_wait=list(waits[i:i + WAIT_LIMIT]), on_update=[])
                else:
                    xsi.on_wait = list(waits[i:i + WAIT_LIMIT])
        nc.sync.drain()
        nc.all_engine_barrier()
        assert self.sems is not None
        popped = nc._tile_sem_poison_stack.pop()
        assert popped is self._sem_poison
        nc.clear_and_free_semaphores(list(self.sems.allocated().values()))
        nc.all_engine_barrier()


def _fix_sync_waits(nc, limit=WAIT_LIMIT):
    """Move excess per-instruction sem waits onto NOPs spliced just
    before the instruction on the same engine."""
    for func in nc.m.functions:
        for bb in func.blocks:
            insts = list(bb.instructions)
            out = []
            changed = False
            for inst in insts:
                si = getattr(inst, "sync_info", None)
                waits = list(si.on_wait) if si is not None and si.on_wait else []
                if len(waits) > limit:
                    changed = True
                    si.on_wait = waits[:limit]
                    extra = waits[limit:]
                    for i in range(0, len(extra), limit):
                        nop = nc.engines[inst.engine].nop()
                        cur = nc.cur_bb.bb
                        assert cur.instructions[-1] is nop.ins
                        cur.instructions.pop()
                        chunk = list(extra[i:i + limit])
                        if nop.ins.sync_info is None:
                            nop.ins.sync_info = mybir.SyncInfo(
                                on_wait=chunk, on_update=[])
                        else:
                            nop.ins.sync_info.on_wait = chunk
                        out.append(nop.ins)
                out.append(inst)
            if changed:
                bb.instructions.clear()
                bb.instructions.extend(out)


def _ap(t, dims, offset=0):
    return bass.AP(tensor=t, offset=offset, ap=[list(d) for d in dims])


def _groups_of(scales, tol=1e-6):
    """Group channel indices by (approximately) equal scale value."""
    groups = []
    for c, s in enumerate(scales):
        for g in groups:
            if abs(g[0] - s) <= tol * max(1.0, abs(s)):
                g[1].append(c)
                break
        else:
            groups.append([float(s), [c]])
    return groups


def _build(key, repeats=1):
    (g1_key, col_key) = key
    g1_groups = [(s, list(cols)) for s, cols in g1_key]   # stage-1: scale -> chans of {0,1}
    cols = [(stat, s) for stat, s in col_key]             # stage-3 column list (stat, scale)
    J = len(cols)

    nc = bass.Bass('TRN2', target_bir_lowering=False, debug=False)

    def din(name, shape):
        return nc.dram_tensor(name, list(shape), F32, kind="ExternalInput")

    xc_e = din("xc", (2, N))
    yc_e = din("yc", (N,))
    gi_e = din("gi", (GRID,))
    gj_e = din("gj", (GRID,))
    gih_e = din("gih", (16,))
    wproj_e = din("wproj", (2, CIN))
    bin_e = din("bin", (CIN, 1))
    wc1_e = din("wc1", (5 * CIN, 5, CHID))
    wc2_e = din("wc2", (5 * CHID, 5, CHID))
    wc3_e = din("wc3", (5 * CHID, 5, COUT))
    b1_e = din("b1", (CHID, 1))
    b2_e = din("b2", (CHID, 1))
    b3_e = din("b3", (COUT, 1))
    wcols_e = din("wcols", (CIN, J))
    res_e = nc.dram_tensor("res", [2, 16, GRID], F32, kind="ExternalOutput")

    with _TC(nc) as tc, ExitStack() as es:
        const = es.enter_context(tc.tile_pool(name="const", bufs=1))
        work = es.enter_context(tc.tile_pool(name="work", bufs=2))
        big = es.enter_context(tc.tile_pool(name="big", bufs=2))
        psum = es.enter_context(tc.tile_pool(name="psum", bufs=2, space="PSUM"))
        dstage = es.enter_context(tc.tile_pool(name="dstage", bufs=2, space="DRAM"))

        # ---- constants (loaded once) ----
        gib = const.tile([128, GRID], F32)
        nc.sync.dma_start(out=gib[:], in_=_ap(gi_e, [[0, 128], [1, GRID]]))
        gjb = const.tile([128, GRID], F32)
        nc.sync.dma_start(out=gjb[:], in_=_ap(gj_e, [[0, 128], [1, GRID]]))
        gihb = const.tile([GRID, 16], F32)
        nc.sync.dma_start(out=gihb[:], in_=_ap(gih_e, [[0, GRID], [1, 16]]))
        gi_c = const.tile([GRID, 1], F32)
        nc.sync.dma_start(out=gi_c[:], in_=_ap(gi_e, [[1, GRID], [0, 1]]))
        gj_c = const.tile([GRID, 1], F32)
        nc.sync.dma_start(out=gj_c[:], in_=_ap(gj_e, [[1, GRID], [0, 1]]))
        xcs = const.tile([128, 2, 2], F32)
        for k in range(2):
            nc.sync.dma_start(out=xcs[:, k, :],
                              in_=_ap(xc_e, [[1, 128], [N, 2]], offset=128 * k))
        ycs = const.tile([128, 2], F32)
        for k in range(2):
            nc.sync.dma_start(out=ycs[:, k:k + 1],
                              in_=_ap(yc_e, [[1, 128], [0, 1]], offset=128 * k))
        wproj_sb = const.tile([2, CIN], F32)
        nc.sync.dma_start(out=wproj_sb[:], in_=wproj_e.ap())
        bin_sb = const.tile([CIN, 1], F32)
        nc.sync.dma_start(out=bin_sb[:], in_=bin_e.ap())
        wc1_sb = const.tile([5 * CIN, 5, CHID], F32R)
        nc.sync.dma_start(out=wc1_sb[:], in_=wc1_e.ap().bitcast(F32R))
        wc2a_sb = const.tile([96, 5, CHID], F32R)
        nc.sync.dma_start(out=wc2a_sb[:], in_=wc2_e.ap()[0:96].bitcast(F32R))
        wc2b_sb = const.tile([64, 5, CHID], F32R)
        nc.sync.dma_start(out=wc2b_sb[:], in_=wc2_e.ap()[96:160].bitcast(F32R))
        wc3a_sb = const.tile([96, 5, COUT], F32R)
        nc.sync.dma_start(out=wc3a_sb[:], in_=wc3_e.ap()[0:96].bitcast(F32R))
        wc3b_sb = const.tile([64, 5, COUT], F32R)
        nc.sync.dma_start(out=wc3b_sb[:], in_=wc3_e.ap()[96:160].bitcast(F32R))
        b1_sb = const.tile([CHID, 1], F32)
        nc.sync.dma_start(out=b1_sb[:], in_=b1_e.ap())
        b2_sb = const.tile([CHID, 1], F32)
        nc.sync.dma_start(out=b2_sb[:], in_=b2_e.ap())
        b3_sb = const.tile([COUT, 1], F32)
        nc.sync.dma_start(out=b3_sb[:], in_=b3_e.ap())
        wcols_sb = const.tile([CIN, J], F32)
        nc.sync.dma_start(out=wcols_sb[:], in_=wcols_e.ap())
        eps_sb = const.tile([GRID, 1], F32)
        nc.vector.memset(eps_sb[:], 1e-8)

        def build_E(out_sl, in0, scal, coef):
            p, f = out_sl.shape[0], out_sl.shape[-1]
            t = work.tile([p, f], F32, tag="eb_t")
            nc.vector.tensor_scalar_sub(t[:], in0, scal)
            t2 = work.tile([p, f], F32, tag="eb_t2")
            nc.vector.tensor_mul(t2[:], t[:], t[:])
            nc.scalar.activation(out_sl, t2[:], AF.Exp, bias=0.0, scale=coef)

        # stage-3 E/F per unique scale
        u3 = []
        for _, s in cols:
            if not any(abs(s - us) <= 1e-6 * max(1.0, abs(s)) for us in u3):
                u3.append(s)

        for rep in range(repeats):
            # ================= stage 1 =================
            ps_s1 = psum.tile([GRID, GRID, 2], F32, tag="ps_small")
            nmm = 2 * len(g1_groups)
            mi = 0
            for sc, chans in g1_groups:
                coef = -0.5 / (sc * sc)
                e1x = work.tile([128, 2, GRID], F32, tag="e1x")
                e1y = work.tile([128, 2, GRID], F32, tag="e1y")
                T = work.tile([128, 2, GRID, 2], F32, tag="T")
                for k in range(2):
                    build_E(e1x[:, k, :], gib[:], xcs[:, k, 0:1], coef)
                    build_E(e1y[:, k, :], gjb[:], xcs[:, k, 1:2], coef)
                    for c in chans:
                        if c == 0:
                            nc.vector.tensor_copy(out=T[:, k, :, 0], in_=e1y[:, k, :])
                        else:
                            nc.vector.tensor_scalar_mul(
                                T[:, k, :, 1], e1y[:, k, :], ycs[:, k:k + 1])
                c0, c1n = min(chans), max(chans) + 1
                for k in range(2):
                    nc.tensor.matmul(
                        out=ps_s1[:, :, c0:c1n], lhsT=e1x[:, k, :],
                        rhs=T[:, k, :, c0:c1n],
                        start=(mi % 2 == 0), stop=(mi % 2 == 1))
                    mi += 1
            # density normalize -> V1 (32 i, 32 j, 2)
            dsb = work.tile([GRID, GRID], F32, tag="dsb")
            nc.scalar.activation(dsb[:], ps_s1[:, :, 0], AF.Identity,
                                 bias=eps_sb[:], scale=1.0)
            rec = work.tile([GRID, GRID], F32, tag="rec")
            nc.vector.reciprocal(rec[:], dsb[:])
            V1 = work.tile([GRID, GRID, 2], F32, tag="V1")
            nc.scalar.copy(V1[:, :, 0], ps_s1[:, :, 0])
            nc.vector.tensor_mul(V1[:, :, 1], ps_s1[:, :, 1], rec[:])
            # remap (i,(j,c)) -> (c,(i,j)) via DRAM
            vdr = dstage.tile([2048], F32, tag="vdr")
            nc.sync.dma_start(
                out=_ap(vdr.tensor, [[GRID, GRID], [1, GRID], [1024, 2]]),
                in_=V1[:])
            V2 = work.tile([2, GRID, GRID], F32, tag="V2")
            nc.sync.dma_start(
                out=V2[:],
                in_=_ap(vdr.tensor, [[1024, 2], [GRID, GRID], [1, GRID]]))
            # projection to 16ch + sigmoid -> hsig (16, 32, 32)
            ps_hp = psum.tile([CIN, GRID, GRID], F32, tag="ps_wide")
            for ih in range(2):
                nc.tensor.matmul(
                    out=ps_hp[:, ih * 16:ih * 16 + 16, :], lhsT=wproj_sb[:],
                    rhs=V2[:, ih * 16:ih * 16 + 16, :], start=True, stop=True)
            hsig = big.tile([CIN, GRID, GRID], F32, tag="hsig")
            nc.scalar.activation(hsig[:], ps_hp[:], AF.Sigmoid,
                                 bias=bin_sb[:], scale=1.0)

            # ================= stage 2: 3 convs =================
            def pack(dst_list, src, cch):
                # dst_list: [(tile, di_base)] covering di 0..4 split by rows
                for t, _ in dst_list:
                    nc.vector.memset(t[:].bitcast(F32), 0.0)
                engs = [nc.sync, nc.scalar, nc.sync, nc.scalar, nc.sync]
                for di in range(5):
                    lo = max(0, 2 - di)
                    hi = min(GRID, 34 - di)
                    slo = lo + di - 2
                    for t, base in dst_list:
                        npart = t.shape[0]
                        if base <= di * cch < base + npart:
                            po = di * cch - base
                            engs[di].dma_start(
                                out=t[po:po + cch, lo:hi, 2:34],
                                in_=src[:, slo:slo + hi - lo, :].bitcast(F32R))
                            break

            def conv(parts, wcs, cout, src_bias, func, out_t):
                for ih in range(2):
                    ps = psum.tile([cout, 16, GRID], F32, tag="ps_conv")
                    nm = 5 * len(parts)
                    i = 0
                    for P, wc in zip(parts, wcs):
                        for dx in range(5):
                            nc.tensor.matmul(
                                out=ps[:],
                                lhsT=wc[:, dx, :],
                                rhs=P[:, ih * 16:ih * 16 + 16, dx:dx + GRID],
                                start=(i == 0), stop=(i == nm - 1))
                            i += 1
                    nc.scalar.activation(out_t[:, ih * 16:ih * 16 + 16, :],
                                         ps[:], func, bias=src_bias[:], scale=1.0)

            P1 = big.tile([5 * CIN, GRID, 36], F32R, tag="P1")
            pack([(P1, 0)], hsig, CIN)
            r1 = big.tile([CHID, GRID, GRID], F32, tag="r1")
            conv([P1], [wc1_sb], CHID, b1_sb, AF.Relu, r1)

            P2a = big.tile([96, GRID, 36], F32R, tag="P2a")
            P2b = big.tile([64, GRID, 36], F32R, tag="P2b")
            pack([(P2a, 0), (P2b, 96)], r1, CHID)
            r2 = big.tile([CHID, GRID, GRID], F32, tag="r2")
            conv([P2a, P2b], [wc2a_sb, wc2b_sb], CHID, b2_sb, AF.Relu, r2)

            P3a = big.tile([96, GRID, 36], F32R, tag="P3a")
            P3b = big.tile([64, GRID, 36], F32R, tag="P3b")
            pack([(P3a, 0), (P3b, 96)], r2, CHID)
            hs3 = big.tile([COUT, GRID, GRID], F32, tag="hs3")
            conv([P3a, P3b], [wc3a_sb, wc3b_sb], COUT, b3_sb, AF.Identity, hs3)

            # ================= stage 3 =================
            ps_H = psum.tile([J, 1024], F32, tag="ps_wide")
            for ih in range(2):
                nc.tensor.matmul(
                    out=ps_H[:, ih * 512:ih * 512 + 512], lhsT=wcols_sb[:],
                    rhs=hs3[:, ih * 16:ih * 16 + 16, :], start=True, stop=True)
            Hsb = work.tile([J, 1024], F32, tag="Hsb")
            nc.scalar.copy(Hsb[:], ps_H[:])
            hdr = dstage.tile([J * 1024], F32, tag="hdr")
            nc.sync.dma_start(
                out=_ap(hdr.tensor, [[1024, J], [1, 1024]]), in_=Hsb[:])
            H2 = work.tile([GRID, J, GRID], F32, tag="H2")
            nc.sync.dma_start(
                out=H2[:],
                in_=_ap(hdr.tensor, [[GRID, GRID], [1024, J], [1, GRID]]))

            # E/F per unique scale
            Eih = {}
            Fm = {}
            for s in u3:
                coef = -0.5 / (s * s)
                e = work.tile([GRID, 16], F32, tag=f"eih{u3.index(s)}")
                build_E(e[:], gihb[:], gi_c[:], coef)
                f = work.tile([GRID, GRID], F32, tag=f"fm{u3.index(s)}")
                build_E(f[:], gjb[0:GRID, :], gj_c[:], coef)
                Eih[s] = e
                Fm[s] = f

            def find_u3(s):
                for us in u3:
                    if abs(s - us) <= 1e-6 * max(1.0, abs(s)):
                        return us
                raise AssertionError

            # step A: contract i' per contiguous same-scale col run
            ps_R = psum.tile([16, J, GRID], F32, tag="ps_small")
            c = 0
            while c < J:
                s = find_u3(cols[c][1])
                e = c
                while e < J and find_u3(cols[e][1]) is s:
                    e += 1
                nc.tensor.matmul(out=ps_R[:, c:e, :], lhsT=Eih[s][:],
                                 rhs=H2[:, c:e, :], start=True, stop=True)
                c = e
            Rsb = work.tile([16, J, GRID], F32, tag="Rsb")
            nc.scalar.copy(Rsb[:], ps_R[:])
            rdr = dstage.tile([16 * J * GRID], F32, tag="rdr")
            nc.sync.dma_start(
                out=_ap(rdr.tensor, [[GRID, 16], [GRID * 16, J], [1, GRID]]),
                in_=Rsb[:])
            Rt = work.tile([GRID, J, 16], F32, tag="Rt")
            nc.sync.dma_start(
                out=Rt[:],
                in_=_ap(rdr.tensor, [[1, GRID], [GRID * 16, J], [GRID, 16]]))

            # step B: contract j', accumulate per stat
            ps_O = psum.tile([GRID, 2, 16], F32, tag="ps_small")
            seen = [0, 0]
            nstat = [sum(1 for st, _ in cols if st == s) for s in range(2)]
            for jc, (stat, s) in enumerate(cols):
                us = find_u3(s)
                nc.tensor.matmul(
                    out=ps_O[:, stat, :], lhsT=Fm[us][:], rhs=Rt[:, jc, :],
                    start=(seen[stat] == 0), stop=(seen[stat] == nstat[stat] - 1))
                seen[stat] += 1
            o_sb = work.tile([GRID, 2, 16], F32, tag="osb")
            nc.scalar.copy(o_sb[:], ps_O[:])
            nc.sync.dma_start(
                out=_ap(res_e, [[1, GRID], [16 * GRID, 2], [GRID, 16]]),
                in_=o_sb[:])

    _fix_sync_waits(nc)
    return nc


_cache = {}


def _prep(inputs):
    x = np.asarray(inputs['x'], np.float32)
    y = np.asarray(inputs['y'], np.float32)
    x_grid = np.asarray(inputs['x_grid'], np.float32)
    gi = np.ascontiguousarray(x_grid[::GRID, 0])
    gj = np.ascontiguousarray(x_grid[:GRID, 1])
    mesh = np.stack(np.meshgrid(gi, gj, indexing='ij'), -1).reshape(-1, 2)
    assert np.allclose(mesh, x_grid, atol=1e-6), "x_grid is not a product grid"

    s_in = np.exp(np.asarray(inputs['sigma_in'], np.float64))
    g1 = _groups_of(s_in)
    g1_key = tuple((s, tuple(cs)) for s, cs in g1)

    s_m = np.exp(np.asarray(inputs['sigma_m'], np.float64))
    s_s = np.exp(np.asarray(inputs['sigma_s'], np.float64))
    cols = []          # (stat, scale, weight-vector)
    for stat, (sv, w) in enumerate(
            [(s_m, inputs['W_m'][0]), (s_s, inputs['W_s'][0])]):
        for s, cs in _groups_of(sv):
            wv = np.zeros(CIN, np.float32)
            for cc in cs:
                wv[cc] = w[cc]
            cols.append((stat, s, wv))
    col_key = tuple((st, s) for st, s, _ in cols)
    wcols = np.stack([wv for _, _, wv in cols], 1)        # (16, J)

    wc1 = np.ascontiguousarray(
        np.asarray(inputs['conv_W1'], np.float32).transpose(3, 1, 2, 0)
        .reshape(5 * CIN, 5, CHID))
    wc2 = np.ascontiguousarray(
        np.asarray(inputs['conv_W2'], np.float32).transpose(3, 1, 2, 0)
        .reshape(5 * CHID, 5, CHID))
    wc3 = np.ascontiguousarray(
        np.asarray(inputs['conv_W3'], np.float32).transpose(3, 1, 2, 0)
        .reshape(5 * CHID, 5, COUT))

    shared = {
        'gi': gi, 'gj': gj,
        'wproj': np.ascontiguousarray(np.asarray(inputs['W_in'], np.float32).T),
        'bin': np.asarray(inputs['b_in'], np.float32).reshape(CIN, 1),
        'wc1': wc1, 'wc2': wc2, 'wc3': wc3,
        'b1': np.asarray(inputs['conv_b1'], np.float32).reshape(CHID, 1),
        'b2': np.asarray(inputs['conv_b2'], np.float32).reshape(CHID, 1),
        'b3': np.asarray(inputs['conv_b3'], np.float32).reshape(COUT, 1),
        'wcols': np.ascontiguousarray(wcols),
    }
    in_maps = []
    for c in range(NCORES):
        b, h = c // 2, c % 2
        m = dict(shared)
        m['xc'] = np.ascontiguousarray(x[b].T)
        m['yc'] = np.ascontiguousarray(y[b, :, 0])
        m['gih'] = np.ascontiguousarray(gi[16 * h:16 * h + 16])
        in_maps.append(m)
    key = (g1_key, col_key)
    return key, in_maps


def _assemble(results, inputs):
    b_m = float(np.asarray(inputs['b_m']).reshape(-1)[0])
    b_s = float(np.asarray(inputs['b_s']).reshape(-1)[0])
    out = np.zeros((B, 2 * GRID * GRID), np.float32)
    for c in range(NCORES):
        b, h = c // 2, c % 2
        r = results[c]["res"]                      # (2, 16, 32)
        sl = slice(16 * h * GRID, (16 * h + 16) * GRID)
        out[b, 0:1024][sl] = (r[0] + b_m).reshape(-1)
        out[b, 1024:2048][sl] = (r[1] + b_s).reshape(-1)
    return out


def get_program(key, repeats=1):
    ck = (key, repeats)
    if ck not in _cache:
        _cache[ck] = _build(key, repeats)
    return _cache[ck]


def run(inputs, repeats=1):
    key, in_maps = _prep(inputs)
    nc = get_program(key, repeats)
    res = run_bass_kernel_spmd(nc, in_maps, list(range(NCORES)))
    return _assemble(res.results, inputs)


def kernel(**inputs):
    return run(inputs, repeats=1)



# revision 2
# speedup vs baseline: 102.7315x; 102.7315x over previous
"""ConvCNP2D Trainium2 kernel.

Strategy (8 cores, uniform SPMD program; per-core data selects work):
  core c -> batch b = c//2, grid i-half h = c%2.

The grid is a product grid (meshgrid of two 1D linspaces), so every RBF
set-conv separates:  K[n,m] = E1x[n,i_m] * E1y[n,j_m]   (stage 1) and
K2[n,m] = E[i_n,i_m] * F[j_n,j_m] with E,F only 32x32 (stage 3). This
removes the (b,n,m,C) materialization entirely: stage 1 is one 256-
contraction matmul, stage 3 is two 32-contraction matmuls on each side
of a 16-channel projection that is folded in FIRST (W_m/W_s are (1,16),
so project h -> (2,1024) before the grid transforms).

The 5x5 convs run as 5 accumulated matmuls over (ci*5row-shift) packed
partitions; the packs are pure DMA row-shifted copies (f32r views) so
TensorE streams at 1 cyc/row (f32r) instead of fp32's 4.

Partition-crossing reshapes (free dim -> partition dim) ride through
small DRAM staging buffers (DMA-only, ~KBs).
"""
import sys

for _p in ('/opt/trn_rl_repo', '/root/.axon_site/_ro/trn_rl_repo'):
    if _p not in sys.path:
        sys.path.append(_p)

import numpy as np
import concourse.bass as bass
import concourse.tile as tile
from concourse import mybir
from concourse.bass_utils import run_bass_kernel_spmd
from concourse.vector_clock import ScopedClock
from contextlib import ExitStack

F32 = mybir.dt.float32
F32R = mybir.dt.float32r
AF = mybir.ActivationFunctionType

GRID, B, N, CIN, CHID, COUT = 32, 4, 256, 16, 32, 16
NCORES = 8
WAIT_LIMIT = 1


class _TC(tile.TileContext):
    """This walrus build accepts only 1 sync wait per instruction; split
    the end-of-kernel drain's global-clock waits across NOPs."""

    def _drain_and_barrier(self, tick_clock, wait_clock):
        nc = self.nc
        collector = nc.sync.nop()
        wait_clock.add_sem_waits(
            collector.ins, ScopedClock({None: tick_clock.global_clock}))
        si = collector.ins.sync_info
        waits = list(si.on_wait) if si and si.on_wait else []
        if len(waits) > WAIT_LIMIT:
            si.on_wait = waits[:WAIT_LIMIT]
            for i in range(WAIT_LIMIT, len(waits), WAIT_LIMIT):
                extra = nc.sync.nop()
                xsi = extra.ins.sync_info
                if xsi is None:
                    extra.ins.sync_info = mybir.SyncInfo(
                        on_wait=list(waits[i:i + WAIT_LIMIT]), on_update=[])
                else:
                    xsi.on_wait = list(waits[i:i + WAIT_LIMIT])
        nc.sync.drain()
        nc.all_engine_barrier()
        assert self.sems is not None
        popped = nc._tile_sem_poison_stack.pop()
        assert popped is self._sem_poison
        nc.clear_and_free_semaphores(list(self.sems.allocated().values()))
        nc.all_engine_barrier()


def _fix_sync_waits(nc, limit=WAIT_LIMIT):
    """Move excess per-instruction sem waits onto NOPs spliced just
    before the instruction on the same engine."""
    for func in nc.m.functions:
        for bb in func.blocks:
            insts = list(bb.instructions)
            out = []
            changed = False
            for inst in insts:
                si = getattr(inst, "sync_info", None)
                waits = list(si.on_wait) if si is not None and si.on_wait else []
                if len(waits) > limit:
                    changed = True
                    si.on_wait = waits[:limit]
                    extra = waits[limit:]
                    for i in range(0, len(extra), limit):
                        nop = nc.engines[inst.engine].nop()
                        cur = nc.cur_bb.bb
                        assert cur.instructions[-1] is nop.ins
                        cur.instructions.pop()
                        chunk = list(extra[i:i + limit])
                        if nop.ins.sync_info is None:
                            nop.ins.sync_info = mybir.SyncInfo(
                                on_wait=chunk, on_update=[])
                        else:
                            nop.ins.sync_info.on_wait = chunk
                        out.append(nop.ins)
                out.append(inst)
            if changed:
                bb.instructions.clear()
                bb.instructions.extend(out)


def _ap(t, dims, offset=0):
    return bass.AP(tensor=t, offset=offset, ap=[list(d) for d in dims])


def _groups_of(scales, tol=1e-6):
    """Group channel indices by (approximately) equal scale value."""
    groups = []
    for c, s in enumerate(scales):
        for g in groups:
            if abs(g[0] - s) <= tol * max(1.0, abs(s)):
                g[1].append(c)
                break
        else:
            groups.append([float(s), [c]])
    return groups


def _build(key, repeats=1):
    (g1_key, col_key) = key
    g1_groups = [(s, list(cols)) for s, cols in g1_key]   # stage-1: scale -> chans of {0,1}
    cols = [(stat, s) for stat, s in col_key]             # stage-3 column list (stat, scale)
    J = len(cols)

    nc = bass.Bass('TRN2', target_bir_lowering=False, debug=False)

    def din(name, shape):
        return nc.dram_tensor(name, list(shape), F32, kind="ExternalInput")

    xc_e = din("xc", (2, N))
    yc_e = din("yc", (N,))
    gi_e = din("gi", (GRID,))
    gj_e = din("gj", (GRID,))
    gih_e = din("gih", (16,))
    wproj_e = din("wproj", (2, CIN))
    bin_e = din("bin", (CIN, 1))
    wc1_e = din("wc1", (5 * CIN, 5, CHID))
    wc2_e = din("wc2", (5 * CHID, 5, CHID))
    wc3_e = din("wc3", (5 * CHID, 5, COUT))
    b1_e = din("b1", (CHID, 1))
    b2_e = din("b2", (CHID, 1))
    b3_e = din("b3", (COUT, 1))
    wcols_e = din("wcols", (CIN, J))
    res_e = nc.dram_tensor("res", [2, 16, GRID], F32, kind="ExternalOutput")

    with _TC(nc) as tc, ExitStack() as es:
        const = es.enter_context(tc.tile_pool(name="const", bufs=1))
        work = es.enter_context(tc.tile_pool(name="work", bufs=2))
        big = es.enter_context(tc.tile_pool(name="big", bufs=2))
        psum = es.enter_context(tc.tile_pool(name="psum", bufs=2, space="PSUM"))
        dstage = es.enter_context(tc.tile_pool(name="dstage", bufs=2, space="DRAM"))

        # ---- constants (loaded once) ----
        gib = const.tile([128, GRID], F32)
        nc.sync.dma_start(out=gib[:], in_=_ap(gi_e, [[0, 128], [1, GRID]]))
        gjb = const.tile([128, GRID], F32)
        nc.sync.dma_start(out=gjb[:], in_=_ap(gj_e, [[0, 128], [1, GRID]]))
        gihb = const.tile([GRID, 16], F32)
        nc.sync.dma_start(out=gihb[:], in_=_ap(gih_e, [[0, GRID], [1, 16]]))
        gi_c = const.tile([GRID, 1], F32)
        nc.sync.dma_start(out=gi_c[:], in_=_ap(gi_e, [[1, GRID], [0, 1]]))
        gj_c = const.tile([GRID, 1], F32)
        nc.sync.dma_start(out=gj_c[:], in_=_ap(gj_e, [[1, GRID], [0, 1]]))
        xcs = const.tile([128, 2, 2], F32)
        for k in range(2):
            nc.sync.dma_start(out=xcs[:, k, :],
                              in_=_ap(xc_e, [[1, 128], [N, 2]], offset=128 * k))
        ycs = const.tile([128, 2], F32)
        for k in range(2):
            nc.sync.dma_start(out=ycs[:, k:k + 1],
                              in_=_ap(yc_e, [[1, 128], [0, 1]], offset=128 * k))
        wproj_sb = const.tile([2, CIN], F32)
        nc.sync.dma_start(out=wproj_sb[:], in_=wproj_e.ap())
        bin_sb = const.tile([CIN, 1], F32)
        nc.sync.dma_start(out=bin_sb[:], in_=bin_e.ap())
        wc1_sb = const.tile([5 * CIN, 5, CHID], F32R)
        nc.sync.dma_start(out=wc1_sb[:], in_=wc1_e.ap().bitcast(F32R))
        wc2a_sb = const.tile([96, 5, CHID], F32R)
        nc.sync.dma_start(out=wc2a_sb[:], in_=wc2_e.ap()[0:96].bitcast(F32R))
        wc2b_sb = const.tile([64, 5, CHID], F32R)
        nc.sync.dma_start(out=wc2b_sb[:], in_=wc2_e.ap()[96:160].bitcast(F32R))
        wc3a_sb = const.tile([96, 5, COUT], F32R)
        nc.sync.dma_start(out=wc3a_sb[:], in_=wc3_e.ap()[0:96].bitcast(F32R))
        wc3b_sb = const.tile([64, 5, COUT], F32R)
        nc.sync.dma_start(out=wc3b_sb[:], in_=wc3_e.ap()[96:160].bitcast(F32R))
        b1_sb = const.tile([CHID, 1], F32)
        nc.sync.dma_start(out=b1_sb[:], in_=b1_e.ap())
        b2_sb = const.tile([CHID, 1], F32)
        nc.sync.dma_start(out=b2_sb[:], in_=b2_e.ap())
        b3_sb = const.tile([COUT, 1], F32)
        nc.sync.dma_start(out=b3_sb[:], in_=b3_e.ap())
        wcols_sb = const.tile([CIN, J], F32)
        nc.sync.dma_start(out=wcols_sb[:], in_=wcols_e.ap())
        eps_sb = const.tile([GRID, 1], F32)
        nc.vector.memset(eps_sb[:], 1e-8)

        def build_E(out_sl, in0, scal, coef):
            p, f = out_sl.shape[0], out_sl.shape[-1]
            t = work.tile([p, f], F32, tag="eb_t")
            nc.vector.tensor_scalar_sub(t[:], in0, scal)
            t2 = work.tile([p, f], F32, tag="eb_t2")
            nc.vector.tensor_mul(t2[:], t[:], t[:])
            nc.scalar.activation(out_sl, t2[:], AF.Exp, bias=0.0, scale=coef)

        # stage-3 E/F per unique scale
        u3 = []
        for _, s in cols:
            if not any(abs(s - us) <= 1e-6 * max(1.0, abs(s)) for us in u3):
                u3.append(s)

        from contextlib import nullcontext
        with (tc.For_i(0, repeats) if repeats > 1 else nullcontext()):
            # ================= stage 1 =================
            ps_s1 = psum.tile([GRID, GRID, 2], F32, tag="ps_small")
            nmm = 2 * len(g1_groups)
            mi = 0
            for sc, chans in g1_groups:
                coef = -0.5 / (sc * sc)
                e1x = work.tile([128, 2, GRID], F32, tag="e1x")
                e1y = work.tile([128, 2, GRID], F32, tag="e1y")
                T = work.tile([128, 2, GRID, 2], F32, tag="T")
                for k in range(2):
                    build_E(e1x[:, k, :], gib[:], xcs[:, k, 0:1], coef)
                    build_E(e1y[:, k, :], gjb[:], xcs[:, k, 1:2], coef)
                    for c in chans:
                        if c == 0:
                            nc.vector.tensor_copy(out=T[:, k, :, 0], in_=e1y[:, k, :])
                        else:
                            nc.vector.tensor_scalar_mul(
                                T[:, k, :, 1], e1y[:, k, :], ycs[:, k:k + 1])
                c0, c1n = min(chans), max(chans) + 1
                for k in range(2):
                    nc.tensor.matmul(
                        out=ps_s1[:, :, c0:c1n], lhsT=e1x[:, k, :],
                        rhs=T[:, k, :, c0:c1n],
                        start=(mi % 2 == 0), stop=(mi % 2 == 1))
                    mi += 1
            # density normalize -> V1 (32 i, 32 j, 2)
            dsb = work.tile([GRID, GRID], F32, tag="dsb")
            nc.scalar.activation(dsb[:], ps_s1[:, :, 0], AF.Identity,
                                 bias=eps_sb[:], scale=1.0)
            rec = work.tile([GRID, GRID], F32, tag="rec")
            nc.vector.reciprocal(rec[:], dsb[:])
            V1 = work.tile([GRID, GRID, 2], F32, tag="V1")
            nc.scalar.copy(V1[:, :, 0], ps_s1[:, :, 0])
            nc.vector.tensor_mul(V1[:, :, 1], ps_s1[:, :, 1], rec[:])
            # remap (i,(j,c)) -> (c,(i,j)) via DRAM
            vdr = dstage.tile([2048], F32, tag="vdr")
            nc.sync.dma_start(
                out=_ap(vdr.tensor, [[GRID, GRID], [1, GRID], [1024, 2]]),
                in_=V1[:])
            V2 = work.tile([2, GRID, GRID], F32, tag="V2")
            nc.sync.dma_start(
                out=V2[:],
                in_=_ap(vdr.tensor, [[1024, 2], [GRID, GRID], [1, GRID]]))
            # projection to 16ch + sigmoid -> hsig (16, 32, 32)
            ps_hp = psum.tile([CIN, GRID, GRID], F32, tag="ps_wide")
            for ih in range(2):
                nc.tensor.matmul(
                    out=ps_hp[:, ih * 16:ih * 16 + 16, :], lhsT=wproj_sb[:],
                    rhs=V2[:, ih * 16:ih * 16 + 16, :], start=True, stop=True)
            hsig = big.tile([CIN, GRID, GRID], F32, tag="hsig")
            nc.scalar.activation(hsig[:], ps_hp[:], AF.Sigmoid,
                                 bias=bin_sb[:], scale=1.0)

            # ================= stage 2: 3 convs =================
            def pack(dst_list, src, cch):
                # dst_list: [(tile, di_base)] covering di 0..4 split by rows
                for t, _ in dst_list:
                    nc.vector.memset(t[:].bitcast(F32), 0.0)
                engs = [nc.sync, nc.scalar, nc.sync, nc.scalar, nc.sync]
                for di in range(5):
                    lo = max(0, 2 - di)
                    hi = min(GRID, 34 - di)
                    slo = lo + di - 2
                    for t, base in dst_list:
                        npart = t.shape[0]
                        if base <= di * cch < base + npart:
                            po = di * cch - base
                            engs[di].dma_start(
                                out=t[po:po + cch, lo:hi, 2:34],
                                in_=src[:, slo:slo + hi - lo, :].bitcast(F32R))
                            break

            def conv(parts, wcs, cout, src_bias, func, out_t):
                for ih in range(2):
                    ps = psum.tile([cout, 16, GRID], F32, tag="ps_conv")
                    nm = 5 * len(parts)
                    i = 0
                    for P, wc in zip(parts, wcs):
                        for dx in range(5):
                            nc.tensor.matmul(
                                out=ps[:],
                                lhsT=wc[:, dx, :],
                                rhs=P[:, ih * 16:ih * 16 + 16, dx:dx + GRID],
                                start=(i == 0), stop=(i == nm - 1))
                            i += 1
                    nc.scalar.activation(out_t[:, ih * 16:ih * 16 + 16, :],
                                         ps[:], func, bias=src_bias[:], scale=1.0)

            P1 = big.tile([5 * CIN, GRID, 36], F32R, tag="P1")
            pack([(P1, 0)], hsig, CIN)
            r1 = big.tile([CHID, GRID, GRID], F32, tag="r1")
            conv([P1], [wc1_sb], CHID, b1_sb, AF.Relu, r1)

            P2a = big.tile([96, GRID, 36], F32R, tag="P2a")
            P2b = big.tile([64, GRID, 36], F32R, tag="P2b")
            pack([(P2a, 0), (P2b, 96)], r1, CHID)
            r2 = big.tile([CHID, GRID, GRID], F32, tag="r2")
            conv([P2a, P2b], [wc2a_sb, wc2b_sb], CHID, b2_sb, AF.Relu, r2)

            P3a = big.tile([96, GRID, 36], F32R, tag="P3a")
            P3b = big.tile([64, GRID, 36], F32R, tag="P3b")
            pack([(P3a, 0), (P3b, 96)], r2, CHID)
            hs3 = big.tile([COUT, GRID, GRID], F32, tag="hs3")
            conv([P3a, P3b], [wc3a_sb, wc3b_sb], COUT, b3_sb, AF.Identity, hs3)

            # ================= stage 3 =================
            ps_H = psum.tile([J, 1024], F32, tag="ps_wide")
            for ih in range(2):
                nc.tensor.matmul(
                    out=ps_H[:, ih * 512:ih * 512 + 512], lhsT=wcols_sb[:],
                    rhs=hs3[:, ih * 16:ih * 16 + 16, :], start=True, stop=True)
            Hsb = work.tile([J, 1024], F32, tag="Hsb")
            nc.scalar.copy(Hsb[:], ps_H[:])
            hdr = dstage.tile([J * 1024], F32, tag="hdr")
            nc.sync.dma_start(
                out=_ap(hdr.tensor, [[1024, J], [1, 1024]]), in_=Hsb[:])
            H2 = work.tile([GRID, J, GRID], F32, tag="H2")
            nc.sync.dma_start(
                out=H2[:],
                in_=_ap(hdr.tensor, [[GRID, GRID], [1024, J], [1, GRID]]))

            # E/F per unique scale
            Eih = {}
            Fm = {}
            for s in u3:
                coef = -0.5 / (s * s)
                e = work.tile([GRID, 16], F32, tag=f"eih{u3.index(s)}")
                build_E(e[:], gihb[:], gi_c[:], coef)
                f = work.tile([GRID, GRID], F32, tag=f"fm{u3.index(s)}")
                build_E(f[:], gjb[0:GRID, :], gj_c[:], coef)
                Eih[s] = e
                Fm[s] = f

            def find_u3(s):
                for us in u3:
                    if abs(s - us) <= 1e-6 * max(1.0, abs(s)):
                        return us
                raise AssertionError

            # step A: contract i' per contiguous same-scale col run
            ps_R = psum.tile([16, J, GRID], F32, tag="ps_small")
            c = 0
            while c < J:
                s = find_u3(cols[c][1])
                e = c
                while e < J and find_u3(cols[e][1]) is s:
                    e += 1
                nc.tensor.matmul(out=ps_R[:, c:e, :], lhsT=Eih[s][:],
                                 rhs=H2[:, c:e, :], start=True, stop=True)
                c = e
            Rsb = work.tile([16, J, GRID], F32, tag="Rsb")
            nc.scalar.copy(Rsb[:], ps_R[:])
            rdr = dstage.tile([16 * J * GRID], F32, tag="rdr")
            nc.sync.dma_start(
                out=_ap(rdr.tensor, [[GRID, 16], [GRID * 16, J], [1, GRID]]),
                in_=Rsb[:])
            Rt = work.tile([GRID, J, 16], F32, tag="Rt")
            nc.sync.dma_start(
                out=Rt[:],
                in_=_ap(rdr.tensor, [[1, GRID], [GRID * 16, J], [GRID, 16]]))

            # step B: contract j', accumulate per stat
            ps_O = psum.tile([GRID, 2, 16], F32, tag="ps_small")
            seen = [0, 0]
            nstat = [sum(1 for st, _ in cols if st == s) for s in range(2)]
            for jc, (stat, s) in enumerate(cols):
                us = find_u3(s)
                nc.tensor.matmul(
                    out=ps_O[:, stat, :], lhsT=Fm[us][:], rhs=Rt[:, jc, :],
                    start=(seen[stat] == 0), stop=(seen[stat] == nstat[stat] - 1))
                seen[stat] += 1
            o_sb = work.tile([GRID, 2, 16], F32, tag="osb")
            nc.scalar.copy(o_sb[:], ps_O[:])
            nc.sync.dma_start(
                out=_ap(res_e, [[1, GRID], [16 * GRID, 2], [GRID, 16]]),
                in_=o_sb[:])

    _fix_sync_waits(nc)
    return nc


_cache = {}


def _prep(inputs):
    x = np.asarray(inputs['x'], np.float32)
    y = np.asarray(inputs['y'], np.float32)
    x_grid = np.asarray(inputs['x_grid'], np.float32)
    gi = np.ascontiguousarray(x_grid[::GRID, 0])
    gj = np.ascontiguousarray(x_grid[:GRID, 1])
    mesh = np.stack(np.meshgrid(gi, gj, indexing='ij'), -1).reshape(-1, 2)
    assert np.allclose(mesh, x_grid, atol=1e-6), "x_grid is not a product grid"

    s_in = np.exp(np.asarray(inputs['sigma_in'], np.float64))
    g1 = _groups_of(s_in)
    g1_key = tuple((s, tuple(cs)) for s, cs in g1)

    s_m = np.exp(np.asarray(inputs['sigma_m'], np.float64))
    s_s = np.exp(np.asarray(inputs['sigma_s'], np.float64))
    cols = []          # (stat, scale, weight-vector)
    for stat, (sv, w) in enumerate(
            [(s_m, inputs['W_m'][0]), (s_s, inputs['W_s'][0])]):
        for s, cs in _groups_of(sv):
            wv = np.zeros(CIN, np.float32)
            for cc in cs:
                wv[cc] = w[cc]
            cols.append((stat, s, wv))
    col_key = tuple((st, s) for st, s, _ in cols)
    wcols = np.stack([wv for _, _, wv in cols], 1)        # (16, J)

    wc1 = np.ascontiguousarray(
        np.asarray(inputs['conv_W1'], np.float32).transpose(3, 1, 2, 0)
        .reshape(5 * CIN, 5, CHID))
    wc2 = np.ascontiguousarray(
        np.asarray(inputs['conv_W2'], np.float32).transpose(3, 1, 2, 0)
        .reshape(5 * CHID, 5, CHID))
    wc3 = np.ascontiguousarray(
        np.asarray(inputs['conv_W3'], np.float32).transpose(3, 1, 2, 0)
        .reshape(5 * CHID, 5, COUT))

    shared = {
        'gi': gi, 'gj': gj,
        'wproj': np.ascontiguousarray(np.asarray(inputs['W_in'], np.float32).T),
        'bin': np.asarray(inputs['b_in'], np.float32).reshape(CIN, 1),
        'wc1': wc1, 'wc2': wc2, 'wc3': wc3,
        'b1': np.asarray(inputs['conv_b1'], np.float32).reshape(CHID, 1),
        'b2': np.asarray(inputs['conv_b2'], np.float32).reshape(CHID, 1),
        'b3': np.asarray(inputs['conv_b3'], np.float32).reshape(COUT, 1),
        'wcols': np.ascontiguousarray(wcols),
    }
    in_maps = []
    for c in range(NCORES):
        b, h = c // 2, c % 2
        m = dict(shared)
        m['xc'] = np.ascontiguousarray(x[b].T)
        m['yc'] = np.ascontiguousarray(y[b, :, 0])
        m['gih'] = np.ascontiguousarray(gi[16 * h:16 * h + 16])
        in_maps.append(m)
    key = (g1_key, col_key)
    return key, in_maps


def _assemble(results, inputs):
    b_m = float(np.asarray(inputs['b_m']).reshape(-1)[0])
    b_s = float(np.asarray(inputs['b_s']).reshape(-1)[0])
    out = np.zeros((B, 2 * GRID * GRID), np.float32)
    for c in range(NCORES):
        b, h = c // 2, c % 2
        r = results[c]["res"]                      # (2, 16, 32)
        sl = slice(16 * h * GRID, (16 * h + 16) * GRID)
        out[b, 0:1024][sl] = (r[0] + b_m).reshape(-1)
        out[b, 1024:2048][sl] = (r[1] + b_s).reshape(-1)
    return out


def get_program(key, repeats=1):
    ck = (key, repeats)
    if ck not in _cache:
        _cache[ck] = _build(key, repeats)
    return _cache[ck]


def run(inputs, repeats=1):
    key, in_maps = _prep(inputs)
    nc = get_program(key, repeats)
    res = run_bass_kernel_spmd(nc, in_maps, list(range(NCORES)))
    return _assemble(res.results, inputs)


def kernel(**inputs):
    return run(inputs, repeats=1)

